# revision 12
# baseline (speedup 1.0000x reference)
"""Autoregressive LSTM cell (B=256, T=256, D=256, H=1024, O=256) on 8 TRN2 cores.

Compute strategy (unchanged from the tuned baseline): pure data-parallel over
batch (32 rows/core, no collectives in the recurrence). Per step t (256 steps):
    z = x_t @ Wxx + y_{t-1} @ Wxy + h_{t-1} @ Wh     (+b)
    i,f,g,o gates -> c = sig(f)*c + sig(i)*tanh(g); h = sig(o)*tanh(c)
    y = tanh(h @ Wd + bd)
Matmuls are activation-stationary (lhsT = activation^T, bf16 weights stream,
fp32 PSUM + fp32 gate math), 4-way PE column tiling packs 4 batch-32 matmuls,
weight columns host-permuted so gate elementwise ops are partition-aligned,
h/y fed back transposed via PE transpose-mode matmuls, x-part matmuls of step
t+1 software-pipelined into step t. Device time ~1.5 ms for the full
256-step recurrence.

Dispatch strategy (this is where the wall-clock goes): the axon tunnel to the
TRN2 terminal moves ~35-44 MB/s half-duplex, so end-to-end time is dominated
by host<->device bytes, not device compute. This file therefore dispatches
through a cached jit of the same `_bass_exec_p` primitive that
`bass_utils.run_bass_kernel_spmd` lowers to, with three byte-level changes:
  * weights cross the tunnel ONCE as a packed 13.1 MB bf16 shard buffer
    (P("core")) and are all-gathered + 8x-tiled device-side by a tiny XLA jit
    (the stock run_bass_kernel_spmd path re-uploads 8 replicated copies,
    ~100 MB, every call);
  * the donated output buffers (PJRT custom-call outputs must be donated
    parameters) are created on-device by a jitted jnp.zeros instead of
    shipping ~67 MB of host zeros per call;
  * ys is emitted bf16 (DMA from the bf16 feedback cast that already exists
    for the PE transposes), halving the downlink; the host widens to fp32.
The jitted executables are cached at module level, so repeat calls skip the
re-trace + re-lower + NEFF-wrap (~3 s) that run_bass_kernel_spmd pays per
call (it builds a fresh closure every time).
"""

import sys

for p in ("/opt/trn_rl_repo",):
    if p not in sys.path:
        sys.path.insert(0, p)

from contextlib import ExitStack

import numpy as np

import concourse.bacc as bacc
import concourse.bass as bass
import concourse.mybir as mybir
import concourse.tile as tile
from concourse.masks import make_identity

F32 = mybir.dt.float32
BF16 = mybir.dt.bfloat16
AF = mybir.ActivationFunctionType

B, T, D, H, O = 256, 256, 256, 1024, 256
NCORES = 8
BL = B // NCORES  # 32
G4 = 4 * H  # 4096
KX, KY, KH = D // 128, O // 128, H // 128  # 2, 2, 8

W_SHAPES = (("Wxx", (D, G4)), ("Wxy", (O, G4)), ("Wh", (H, G4)),
            ("Wd", (H, O)))
# packed weight matrix [128, WPK_COLS]: twelve [128, 4096] column slabs
# (Wxx k=0..1 | Wxy k=0..1 | Wh k=0..7, slab i = that weight's rows
# [128k : 128k+128]) followed by eight [128, 256] slabs for Wd
WPK_COLS = 12 * G4 + 8 * O  # 51200
WD_COL0 = 12 * G4  # 49152


def gate_perm() -> np.ndarray:
    """Map stored z column position -> original gate column (i,f,g,o order)."""
    perm = np.empty(G4, dtype=np.int64)
    for beta in (0, 1):
        for j in range(4):
            for half in (0, 1):
                gate = (0, 1, 2, 3)[2 * beta + half]
                src = 1024 * gate + 256 * j
                pos = 2048 * beta + 512 * j + 256 * half
                perm[pos : pos + 256] = np.arange(src, src + 256)
    return perm


def _hT_off(c: int) -> int:
    """Column offset of h^T chunk c (channels 128c:128c+128) inside hT_sb."""
    return 128 * (c % 2) + 32 * (c // 2)


def build_nc(T_steps: int = T, use_bias_z: bool = False, use_bias_y: bool = False,
             mm_dt=mybir.dt.bfloat16):
    nc = bacc.Bacc()

    xT_d = nc.declare_dram_parameter("xT", [T_steps, 128, 2 * BL], mm_dt,
                                     isOutput=False)
    Wpk_d = nc.declare_dram_parameter("Wpk", [128, WPK_COLS], mm_dt,
                                      isOutput=False)
    bz_d = by_d = None
    if use_bias_z:
        bz_d = nc.declare_dram_parameter("bz", [128, 1024], F32, isOutput=False)
    if use_bias_y:
        by_d = nc.declare_dram_parameter("by", [BL, O], F32, isOutput=False)
    ys_d = nc.declare_dram_parameter("ys", [BL, T_steps, O], mm_dt,
                                     isOutput=True)

    def mc(ap):
        return ap.bitcast(mm_dt) if ap.dtype != mm_dt else ap

    with tile.TileContext(nc) as tc:
        with ExitStack() as ctx:
            wpool = ctx.enter_context(tc.tile_pool(name="weights", bufs=1))
            state = ctx.enter_context(tc.tile_pool(name="state", bufs=1))
            xpool = ctx.enter_context(tc.tile_pool(name="xin", bufs=3))
            gpool = ctx.enter_context(tc.tile_pool(name="gates", bufs=1))
            hpool = ctx.enter_context(tc.tile_pool(name="hT", bufs=1))
            ypool = ctx.enter_context(tc.tile_pool(name="yt", bufs=1))
            zpsum = ctx.enter_context(tc.tile_pool(name="zps", bufs=2, space="PSUM"))
            ypsum = ctx.enter_context(tc.tile_pool(name="yps", bufs=2, space="PSUM"))
            tpsum = ctx.enter_context(tc.tile_pool(name="tps", bufs=2, space="PSUM"))

            Wxx_sb = wpool.tile([128, KX * G4], mm_dt)
            Wxy_sb = wpool.tile([128, KY * G4], mm_dt)
            Wh_sb = wpool.tile([128, KH * G4], mm_dt)
            Wd_sb = wpool.tile([128, KH * O], mm_dt)
            # Matmult instructions can carry at most ONE sem wait in this
            # lowering; every matmul dependency must resolve to a single DVE
            # sem value. Weight DMAs are therefore "laundered" through
            # in-place DVE copies (one per DMA so each copy waits on one
            # DMA-queue sem only).
            for k in range(KX):
                nc.sync.dma_start(Wxx_sb[:, k * G4 : (k + 1) * G4],
                                  Wpk_d[:, k * G4 : (k + 1) * G4])
                nc.vector.tensor_copy(Wxx_sb[:, k * G4 : (k + 1) * G4],
                                      Wxx_sb[:, k * G4 : (k + 1) * G4])
            for k in range(KY):
                nc.sync.dma_start(Wxy_sb[:, k * G4 : (k + 1) * G4],
                                  Wpk_d[:, (KX + k) * G4 : (KX + k + 1) * G4])
                nc.vector.tensor_copy(Wxy_sb[:, k * G4 : (k + 1) * G4],
                                      Wxy_sb[:, k * G4 : (k + 1) * G4])
            for k in range(KH):
                nc.sync.dma_start(
                    Wh_sb[:, k * G4 : (k + 1) * G4],
                    Wpk_d[:, (KX + KY + k) * G4 : (KX + KY + k + 1) * G4])
                nc.vector.tensor_copy(Wh_sb[:, k * G4 : (k + 1) * G4],
                                      Wh_sb[:, k * G4 : (k + 1) * G4])
                nc.sync.dma_start(
                    Wd_sb[:, k * O : (k + 1) * O],
                    Wpk_d[:, WD_COL0 + k * O : WD_COL0 + (k + 1) * O])
                nc.vector.tensor_copy(Wd_sb[:, k * O : (k + 1) * O],
                                      Wd_sb[:, k * O : (k + 1) * O])
            if use_bias_z:
                bz_sb = wpool.tile([128, 1024], F32)
                nc.sync.dma_start(bz_sb[:], bz_d[:, :])
            if use_bias_y:
                by_sb = wpool.tile([BL, O], F32)
                nc.sync.dma_start(by_sb[:], by_d[:, :])

            # identity for PE transposes (bf16: f32 transpose-mode faults on
            # hw); I64 in both partition halves so the fmap can start at
            # partition 0 or 64 (must match the weights)
            ident = wpool.tile([128, 128], mm_dt)
            make_identity(nc, ident[:])
            nc.vector.tensor_copy(ident[:], ident[:])  # launder Pool dep -> DVE

            # c state, channel(32j+b, n) = 256j + n
            c_sb = state.tile([128, 256], F32)
            nc.gpsimd.memset(c_sb[:], 0.0)

            def emit_z_mms(z_tile, chunks, start, stop):
                nck = len(chunks)
                for ci, (lhsT, wtile, coff) in enumerate(chunks):
                    for beta in range(2):
                        for j in range(4):
                            w_lo = coff + 2048 * beta + 512 * j
                            nc.tensor.matmul(
                                z_tile[32 * j : 32 * (j + 1),
                                       512 * beta : 512 * (beta + 1)],
                                mc(lhsT),
                                mc(wtile[:, w_lo : w_lo + 512]),
                                start=(start and ci == 0),
                                stop=(stop and ci == nck - 1),
                                tile_position=(0, 32 * j),
                                skip_group_check=True,
                            )

            def load_x(t):
                xT_sb = xpool.tile([128, 2 * BL], mm_dt, name="xT_sb")
                nc.sync.dma_start(xT_sb[:], xT_d[t])
                # launder the x DMA-queue sem into the DVE sem
                xr_sb = xpool.tile([128, 2 * BL], mm_dt, name="xr_sb")
                nc.vector.tensor_copy(xr_sb[:], xT_sb[:])
                return [(xr_sb[:, bass.ts(k, BL)], Wxx_sb, k * G4)
                        for k in range(KX)]

            hT_prev = None
            yT_prev = None
            # software pipeline: the x-part of step t+1 is issued during step
            # t, so the in-order PE has independent work while the gate chain
            # (ACT/DVE) of step t runs.
            z_ps = zpsum.tile([128, 1024], F32, name="z_ps")
            emit_z_mms(z_ps, load_x(0), start=True, stop=(T_steps == 1))
            for t in range(T_steps):
                if t > 0:
                    # h first, y last: the y feedback chain (Wd+tanh+cast+
                    # transpose) of step t-1 gets the h-matmul span as slack
                    chunks = [(hT_prev[:, _hT_off(k) : _hT_off(k) + BL], Wh_sb,
                               k * G4) for k in (0, 2, 4, 6, 1, 3, 5, 7)]
                    chunks += [(yT_prev[:, bass.ts(k, BL)], Wxy_sb, k * G4)
                               for k in range(KY)]
                    emit_z_mms(z_ps, chunks, start=False, stop=True)
                if t + 1 < T_steps:
                    z_next = zpsum.tile([128, 1024], F32, name="z_ps")
                    emit_z_mms(z_next, load_x(t + 1), start=True, stop=False)
                else:
                    z_next = None

                # gate math: <=1 PSUM operand per DVE op
                if use_bias_z:
                    nc.vector.tensor_add(z_ps[:, 0:512], z_ps[:, 0:512],
                                         bz_sb[:, 0:512])
                    nc.vector.tensor_add(z_ps[:, 512:1024], z_ps[:, 512:1024],
                                         bz_sb[:, 512:1024])
                # gate chain split into column halves: half 0 finishes ->
                # its transpose + hT copy run while half 1 still computes, so
                # the even hT-chunk matmuls of step t+1 start earlier
                tg_sb = gpool.tile([128, 256], F32, name="tg_sb")
                o_sb = gpool.tile([128, 256], F32, name="o_sb")
                h_stk = gpool.tile([128, 256], mm_dt, name="h_stk")
                tr_ps = tpsum.tile([128, 320], mm_dt, name="tr_ps")
                hT_sb = hpool.tile([128, 256], mm_dt, name="hT_sb")
                for hf in range(2):
                    s = slice(128 * hf, 128 * hf + 128)
                    nc.scalar.activation(tg_sb[:, s], z_ps[:, 512 + 128 * hf :
                                                           640 + 128 * hf],
                                         AF.Tanh)
                    nc.scalar.activation(z_ps[:, s], z_ps[:, s], AF.Sigmoid)
                    nc.vector.tensor_mul(tg_sb[:, s], z_ps[:, s], tg_sb[:, s])
                    nc.scalar.activation(z_ps[:, 256 + 128 * hf : 384 + 128 * hf],
                                         z_ps[:, 256 + 128 * hf : 384 + 128 * hf],
                                         AF.Sigmoid)
                    nc.vector.tensor_mul(c_sb[:, s],
                                         z_ps[:, 256 + 128 * hf : 384 + 128 * hf],
                                         c_sb[:, s])
                    nc.scalar.activation(o_sb[:, s], z_ps[:, 768 + 128 * hf :
                                                          896 + 128 * hf],
                                         AF.Sigmoid)
                    nc.vector.tensor_add(c_sb[:, s], tg_sb[:, s], c_sb[:, s])
                    nc.scalar.activation(tg_sb[:, s], c_sb[:, s], AF.Tanh)
                    nc.vector.tensor_mul(h_stk[:, s], o_sb[:, s], tg_sb[:, s])
                    nc.tensor.transpose(tr_ps[:, s], h_stk[:, s], ident[:])
                    nc.vector.tensor_copy(hT_sb[:, s], tr_ps[:, s])

                # y = tanh(h @ Wd + bd)
                y_ps = ypsum.tile([BL, O], F32, name="y_ps")
                for k in range(KH):
                    nc.tensor.matmul(
                        y_ps[:],
                        mc(hT_sb[:, _hT_off(k) : _hT_off(k) + BL]),
                        mc(Wd_sb[:, k * O : (k + 1) * O]),
                        start=(k == 0),
                        stop=(k == KH - 1),
                    )
                y_sb = ypool.tile([BL, O], F32, name="y_sb", bufs=2)
                if use_bias_y:
                    nc.vector.tensor_add(y_ps[:], y_ps[:], by_sb[:])
                nc.scalar.activation(y_sb[:], y_ps[:], AF.Tanh)
                # cast y for the bf16 PE-transposes (also launders ACT -> DVE);
                # double-buffered since it is also the output-DMA source
                y_bf = ypool.tile([BL, O], mm_dt, name="y_bf", bufs=2)
                nc.vector.tensor_copy(y_bf[:], y_sb[:])
                # ys leaves the device in bf16 (half the tunnel bytes); the
                # host widens back to fp32
                nc.sync.dma_start(ys_d[:, t, :], y_bf[:])

                # y -> yT via 2 PE transposes
                for q in range(2):
                    nc.tensor.transpose(
                        tr_ps[:, 256 + 32 * q : 256 + 32 * (q + 1)],
                        y_bf[0:BL, 128 * q : 128 * (q + 1)],
                        ident[0:32, 0:32],
                    )
                yT_sb = ypool.tile([128, 2 * BL], mm_dt, name="yT_sb")
                nc.vector.tensor_copy(yT_sb[:], tr_ps[:, 256:320])

                hT_prev = hT_sb
                yT_prev = yT_sb
                z_ps = z_next

    nc.compile()
    return nc


def prep_inputs(x, Wx, Wh, b, Wd, bd, T_steps: int = T):
    """Host-side relayout. Returns (xT_global, w_packed, use_bias_z, use_bias_y).

    xT_global: [NCORES*T, 128, 2*BL] bf16 — core c owns rows [c*T:(c+1)*T],
        xT[t, 128k+d, b] = x[c*BL + b, t, 128k + d].
    w_packed: [128, WPK_COLS] bf16 — the packed weight matrix (see build_nc),
        gate columns pre-permuted for the stacked-PSUM z layout.
    """
    import ml_dtypes

    x = np.asarray(x, dtype=np.float32)[:, :T_steps, :]
    Wx = np.asarray(Wx, dtype=np.float32)
    Wh = np.asarray(Wh, dtype=np.float32)
    b = np.asarray(b, dtype=np.float32)
    Wd = np.asarray(Wd, dtype=np.float32)
    bd = np.asarray(bd, dtype=np.float32)

    perm = gate_perm()
    Wxp = np.ascontiguousarray(Wx[:, perm]).astype(ml_dtypes.bfloat16)
    Whp = np.ascontiguousarray(Wh[:, perm]).astype(ml_dtypes.bfloat16)
    Wdb = Wd.astype(ml_dtypes.bfloat16)
    w_packed = np.empty((128, WPK_COLS), dtype=ml_dtypes.bfloat16)
    for k in range(KX):
        w_packed[:, k * G4 : (k + 1) * G4] = Wxp[k * 128 : (k + 1) * 128]
    for k in range(KY):
        w_packed[:, (KX + k) * G4 : (KX + k + 1) * G4] = \
            Wxp[D + k * 128 : D + (k + 1) * 128]
    for k in range(KH):
        w_packed[:, (KX + KY + k) * G4 : (KX + KY + k + 1) * G4] = \
            Whp[k * 128 : (k + 1) * 128]
        w_packed[:, WD_COL0 + k * O : WD_COL0 + (k + 1) * O] = \
            Wdb[k * 128 : (k + 1) * 128]

    # [B, T, D] -> per-core [T, 128, 2*BL], stacked along axis 0
    xc = x.reshape(NCORES, BL, T_steps, KX, 128)       # [8, BL, T, 2, 128]
    xT = xc.transpose(0, 2, 4, 3, 1)                   # [8, T, 128, 2, BL]
    xT_global = np.ascontiguousarray(
        xT.reshape(NCORES * T_steps, 128, KX * BL)).astype(ml_dtypes.bfloat16)

    return xT_global, w_packed, bool(np.any(b)), bool(np.any(bd))


_EXEC = {}


def _get_exec():
    """Build the bass module + cached jitted dispatch callables (once)."""
    if _EXEC:
        return _EXEC

    import jax
    import jax.numpy as jnp
    from jax.sharding import Mesh, NamedSharding, PartitionSpec
    from jax.experimental.shard_map import shard_map
    from concourse import bass2jax
    from concourse.bass2jax import _bass_exec_p, partition_id_tensor

    bass2jax.install_neuronx_cc_hook()

    nc = build_nc(T, False, False)

    partition_name = (nc.partition_id_tensor.name
                      if nc.partition_id_tensor else None)
    in_names, out_names, out_avals = [], [], []
    for alloc in nc.m.functions[0].allocations:
        if not isinstance(alloc, mybir.MemoryLocationSet):
            continue
        name = alloc.memorylocations[0].name
        if alloc.kind == "ExternalInput":
            if name != partition_name:
                in_names.append(name)
        elif alloc.kind == "ExternalOutput":
            out_names.append(name)
            out_avals.append(jax.core.ShapedArray(
                tuple(alloc.tensor_shape), mybir.dt.np(alloc.dtype)))
    assert in_names == ["xT", "Wpk"], in_names
    assert out_names == ["ys"], out_names
    n_params, n_outs = len(in_names), len(out_names)
    in_names_all = list(in_names) + out_names
    if partition_name is not None:
        in_names_all.append(partition_name)

    def _body(*args):
        operands = list(args)
        if partition_name is not None:
            operands.append(partition_id_tensor())
        outs = _bass_exec_p.bind(
            *operands,
            out_avals=tuple(out_avals),
            in_names=tuple(in_names_all),
            out_names=tuple(out_names),
            lowering_input_output_aliases=(),
            sim_require_finite=True,
            sim_require_nnan=True,
            nc=nc,
        )
        return tuple(outs)

    devices = jax.devices()[:NCORES]
    assert len(devices) == NCORES
    mesh = Mesh(np.asarray(devices), ("core",))
    Pc = NamedSharding(mesh, PartitionSpec("core"))
    Pr = NamedSharding(mesh, PartitionSpec())
    # xT sharded by core; weights replicated (they reach the device ONCE,
    # see below); donated ys zeros sharded by core. The neuron runtime
    # rejects (fails LoadExecutable) any stock-XLA executable that computes
    # on a *sharded* operand, so weights must already be replicated here and
    # shard_map hands them through with P() in_specs.
    in_specs = ((PartitionSpec("core"),)
                + (PartitionSpec(),) * (n_params - 1)
                + (PartitionSpec("core"),) * n_outs)
    out_specs = (PartitionSpec("core"),) * n_outs
    donate = tuple(range(n_params, n_params + n_outs))
    sharded = jax.jit(
        shard_map(_body, mesh=mesh, in_specs=in_specs, out_specs=out_specs,
                  check_rep=False),
        donate_argnums=donate,
        keep_unused=True,
    )

    # Weight shards arrive P("core") (1/8th per core = one 13.1 MB tunnel
    # transfer instead of 8 replicated copies). The identity jit is the one
    # sharded->replicated construct the neuron runtime can load (a pure
    # all-gather); the bass kernel consumes the packed matrix directly, so
    # no further on-device reshaping is needed.
    ident = jax.jit(lambda x: x, out_shardings=Pr)

    mkzeros = jax.jit(lambda: jnp.zeros((NCORES * BL, T, O), jnp.bfloat16),
                      out_shardings=Pc)

    _EXEC.update(sharded=sharded, ident=ident, mkzeros=mkzeros,
                 Pc=Pc, jax=jax, nc=nc)
    return _EXEC


def run_once(xT_global: np.ndarray, w_packed: np.ndarray) -> np.ndarray:
    """One full dispatch: host arrays in, full [B, T, O] fp32 out."""
    ex = _get_exec()
    jax = ex["jax"]
    w_dev = jax.device_put(w_packed, ex["Pc"])       # 13.1 MB over the tunnel
    w_full = ex["ident"](w_dev)                      # on-device all-gather
    # The donated output buffer is pure scratch (the kernel DMA-writes every
    # element of ys), so recycle the previous call's fetched output buffer
    # instead of materializing fresh device zeros each call.
    scratch = _EXEC.pop("_scratch", None)
    if scratch is None:
        scratch = ex["mkzeros"]()                    # device-side
    (ys_dev,) = ex["sharded"](xT_global, w_full, scratch)
    ys16 = np.asarray(ys_dev)                        # [B, T, O] bf16
    _EXEC["_scratch"] = ys_dev                       # donate next call
    # widen bf16->fp32 with a bit shift (faster than ml_dtypes astype)
    u32 = ys16.view(np.uint16).astype(np.uint32) << np.uint32(16)
    return u32.view(np.float32)


def _kernel_bias_fallback(x, Wx, Wh, b, Wd, bd):
    """Nonzero-bias path (not exercised by the reference setup): replicate
    inputs per-core and go through the stock spmd runner."""
    import ml_dtypes
    from concourse.bass_utils import run_bass_kernel_spmd

    xT_global, w_packed, ubz, uby = prep_inputs(x, Wx, Wh, b, Wd, bd, T)
    nc = build_nc(T, ubz, uby)
    perm = gate_perm()
    b = np.asarray(b, dtype=np.float32)[perm]
    shared = {"Wpk": w_packed}
    if ubz:
        bz = np.empty((128, 1024), dtype=np.float32)
        for j in range(4):
            for beta in range(2):
                bz[32 * j : 32 * (j + 1), 512 * beta : 512 * (beta + 1)] = \
                    b[2048 * beta + 512 * j : 2048 * beta + 512 * j + 512][None, :]
        shared["bz"] = bz
    if uby:
        shared["by"] = np.broadcast_to(
            np.asarray(bd, dtype=np.float32), (BL, O)).copy()
    in_maps = [{"xT": xT_global[c * T : (c + 1) * T], **shared}
               for c in range(NCORES)]
    res = run_bass_kernel_spmd(nc, in_maps, list(range(NCORES)))
    ys = np.concatenate([np.asarray(res.results[c]["ys"])
                         for c in range(NCORES)], axis=0)
    u32 = ys.view(np.uint16).astype(np.uint32) << np.uint32(16)
    return u32.view(np.float32)


def kernel(x, Wx, Wh, b, Wd, bd):
    if np.any(np.asarray(b)) or np.any(np.asarray(bd)):
        return _kernel_bias_fallback(x, Wx, Wh, b, Wd, bd)
    xT_global, w_packed, _, _ = prep_inputs(x, Wx, Wh, b, Wd, bd, T)
    return run_once(xT_global, w_packed)
